# revision 9
# baseline (speedup 1.0000x reference)
"""Conv1dFFT (truncated-spectrum FFT conv) on 8 trn2 cores — banded matmul form.

Math: out = irfft(trunc(rfft(xp)) * conj(trunc(rfft(wp))))[..., :W] + b on a ring
of size L. Truncation == ring convolution with the Dirichlet kernel D:

    out[n,f,t] = sum_c sum_s w[f,c,s] * P[n,c,t+s] + b[f]
    P[n,c,j]   = sum_tau x[n,c,tau] * D(j - PAD - tau),  D(d) = sin(pi*H*d/L)/(L*sin(pi*d/L))

Structure exploited (per j-parity jp, P columns compacted by parity):
  D = 0.5*delta  (DVE add of 0.5*x, in "xr")
    + odd part   (opposite-parity taus): BANDED dense matmul (3 tau-tiles around
                 the diagonal per 128-col chunk; Toeplitz => the same D data
                 serves every chunk and every core)
                 + rank-8 far-field correction (SVD of the smooth off-band part,
                 shared across chunks via relative-offset U, global V)
    + even part  (same-parity taus, smooth): global rank-8 correction.

P is stored column-parity-split and batch-stacked: per m-tile (2 batch items
A,B) two SBUF tiles  tileA=[Pe_A;Po_A], tileB=[Po_B;Pe_B]  (the e=0/e=1 x row
orders are swapped host-side so every PSUM->SBUF copy is partition-aligned).
Stage 2 contracts the 9 filter taps as 5 K=128 matmuls per output parity
directly on the stacked tiles - no shifted P2 copy needed.

Sharding: pure data-parallel over batch N: 4 items per core.
"""

from contextlib import ExitStack

import numpy as np
import ml_dtypes

import concourse.bass as bass
import concourse.tile as tile
from concourse import bacc, mybir
from concourse.bass_utils import run_bass_kernel_spmd

# ---- problem constants ----
N, C, W = 32, 64, 4096
F, WW = 128, 9
PAD = 4
OUT_W = W - WW + 1 + 2 * PAD                    # 4096
L = W + 2 * PAD + 2 * (WW - 1) + (OUT_W - 1)    # 8215
INIT_HALF = L // 2 + 1
IB = min(INIT_HALF - 1, int(INIT_HALF * 0.5) + 1)
HALF = INIT_HALF - IB
H = 2 * HALF - 1                                # 4105

# ---- scheme constants ----
N_CORES = 8
NPC = N // N_CORES          # 4 batch items per core
JCOLS = 2052                # per-parity compacted P columns used (max j = 4103)
NCH = 16                    # column chunks per parity
CH = 128                    # cols per chunk (last chunk: 132)
CH_LAST = JCOLS - (NCH - 1) * CH   # 132
NSLOT = 19                  # tau tile slots; real tiles 0..15 at slots 2..17
PADS = (0, 1, 18)           # zero slots
BAND = 3                    # near-band tau tiles per chunk: slots [c+1, c+4)
RF = 8                      # far-field rank
RE = 8                      # even-part rank
RELS = list(range(-16, 0)) + list(range(3, 18))  # far relative tile offsets

bf16 = ml_dtypes.bfloat16


def _dirichlet():
    d = np.arange(-(L + 8), L + 9, dtype=np.float64)
    with np.errstate(invalid="ignore", divide="ignore"):
        Dv = np.sin(np.pi * H * d / L) / (L * np.sin(np.pi * d / L))
    Dv[np.abs(d) % L == 0] = H / L

    def DD(dv):
        return Dv[dv + L + 8]

    return DD


_CONSTS = {}


def _consts():
    """Shared (core-independent) constant tensors: dm, Ufar, Vfar, Uev, Vev."""
    if "c" in _CONSTS:
        return _CONSTS["c"]
    DD = _dirichlet()
    rng = np.random.default_rng(0)
    jc = np.arange(CH_LAST)
    p = np.arange(128)

    # near banded D: d = 2jc + 2jp + 251 - 256b - 2p
    dm = np.zeros((128, 2, BAND, CH_LAST), np.float32)          # [p, jp, b, col]
    for jp in (0, 1):
        for b in range(BAND):
            dm[:, jp, b, :] = DD(2 * jc[None, :] + 2 * jp + 251
                                 - 256 * b - 2 * p[:, None])

    # far field: relative matrix SVD -> per-slot padded U (one psum group) +
    # global V (replicated per i-position for the K=32 zero-padded VS form)
    uf = np.zeros((128, 2, 16, 128), np.float32)    # [p, jp, slot-2, 8c+r]
    vf = np.zeros((32, 2, 4, CH_LAST), np.float32)  # [row, jp, i, col] (8i:8i+8 = V)
    for jp in (0, 1):
        Fm = np.zeros((len(RELS) * 128, CH_LAST))
        for i, rho in enumerate(RELS):
            Fm[i * 128:(i + 1) * 128] = DD(2 * jc[None, :] + 2 * jp + 251
                                           - 256 * rho - 2 * p[:, None])
        G = rng.normal(size=(CH_LAST, 4 * RF))
        Q, _ = np.linalg.qr(Fm @ G)
        u, s, vt = np.linalg.svd(Q.T @ Fm, full_matrices=False)
        UU = (Q @ u[:, :RF]).reshape(len(RELS), 128, RF)
        for s_ in range(2, 18):
            for c in range(NCH):
                rho = s_ - (c + 1)
                if rho in RELS:
                    uf[:, jp, s_ - 2, 8 * c:8 * c + 8] = UU[RELS.index(rho)]
        for i in range(4):
            vf[8 * i:8 * i + 8, jp, i, :] = s[:RF, None] * vt[:RF, :]

    # even part (same parity, 0.5*delta removed): global rank-8
    ue = np.zeros((128, 2, 16, RE), np.float32)                 # [p, jp, rt, r]
    ve = np.zeros((RE, 2, JCOLS), np.float32)                   # [r, jp, col]
    for jp in (0, 1):
        jg = 2 * np.arange(JCOLS) + jp
        tg = 2 * np.arange(2048) + jp
        dmat = jg[None, :] - 4 - tg[:, None]
        A = DD(dmat) - 0.5 * (dmat == 0)
        G = rng.normal(size=(A.shape[1], 4 * RE))
        Q, _ = np.linalg.qr(A @ G)
        u, s, vt = np.linalg.svd(Q.T @ A, full_matrices=False)
        ue[:, jp] = (Q @ u[:, :RE]).reshape(16, 128, RE).transpose(1, 0, 2)
        ve[:, jp, :] = s[:RE, None] * vt[:RE, :]

    _CONSTS["c"] = (dm, uf, vf, ue, ve)
    return _CONSTS["c"]


def build_nc():
    dt = mybir.dt.bfloat16
    f32 = mybir.dt.float32
    nc = bacc.Bacc("TRN2", target_bir_lowering=False, debug=False)

    xeo_d = nc.dram_tensor("xeo", [2, 2, 128, 16, 128], dt, kind="ExternalInput")
    xr_d = nc.dram_tensor("xr", [2, 128, 2, JCOLS], dt, kind="ExternalInput")
    dm_d = nc.dram_tensor("dm", [128, 2, BAND, CH_LAST], dt, kind="ExternalInput")
    uf_d = nc.dram_tensor("uf", [128, 2, 16, 128], dt, kind="ExternalInput")
    ue_d = nc.dram_tensor("ue", [128, 2, 16, RE], dt, kind="ExternalInput")
    vf_d = nc.dram_tensor("vf", [32, 2, 4, CH_LAST], dt, kind="ExternalInput")
    ve_d = nc.dram_tensor("ve", [RE, 2, JCOLS], dt, kind="ExternalInput")
    wt_d = nc.dram_tensor("wt", [128, 2, 2, 5, F], dt, kind="ExternalInput")
    b_d = nc.dram_tensor("bias", [128, 1], f32, kind="ExternalInput")
    out_d = nc.dram_tensor("out", [NPC, F, OUT_W], dt, kind="ExternalOutput")

    with tile.TileContext(nc) as tc, ExitStack() as ctx:
        consts = ctx.enter_context(tc.tile_pool(name="consts", bufs=1))
        xpool = ctx.enter_context(tc.tile_pool(name="x", bufs=1))
        spool = ctx.enter_context(tc.tile_pool(name="st", bufs=1))
        tpool = ctx.enter_context(tc.tile_pool(name="t1", bufs=2))
        oapool = ctx.enter_context(tc.tile_pool(name="oa", bufs=3))
        t1ps = ctx.enter_context(tc.tile_pool(name="t1ps", bufs=1, space="PSUM"))
        ps1 = ctx.enter_context(tc.tile_pool(name="ps1", bufs=3, space="PSUM"))
        ps2 = ctx.enter_context(tc.tile_pool(name="ps2", bufs=3, space="PSUM"))

        # ---- constant loads ----
        dm_t = consts.tile([128, 2, BAND, CH_LAST], dt)
        nc.sync.dma_start(out=dm_t[:], in_=dm_d[:])
        uf_t = consts.tile([128, 2, 16, 128], dt)
        nc.sync.dma_start(out=uf_t[:], in_=uf_d[:])
        ue_t = consts.tile([128, 2, 16, RE], dt)
        nc.sync.dma_start(out=ue_t[:], in_=ue_d[:])
        vf_t = consts.tile([32, 2, 4, CH_LAST], dt)
        nc.scalar.dma_start(out=vf_t[:], in_=vf_d[:])
        ve_t = consts.tile([RE, 2, JCOLS], dt)
        nc.scalar.dma_start(out=ve_t[:], in_=ve_d[:])
        btile = consts.tile([128, 1], f32)
        nc.scalar.dma_start(out=btile[:], in_=b_d[:])

        # ---- x loads: xeo[m][e] [128(tau), NSLOT, 128(row)], pads zeroed ----
        xeo_t = {}
        for m in range(2):
            for e in (1, 0):
                t = xpool.tile([128, NSLOT, 128], dt, name=f"xeo{m}{e}",
                               tag=f"xeo{m}{e}")
                nc.gpsimd.memset(t[:, 0:2, :], 0.0)
                nc.gpsimd.memset(t[:, 18:19, :], 0.0)
                nc.sync.dma_start(out=t[:, 2:18, :], in_=xeo_d[m, e])
                xeo_t[(m, e)] = t
        xr_t = []
        for m in range(2):
            t = xpool.tile([128, 2, JCOLS], dt, name=f"xr{m}", tag=f"xr{m}")
            nc.scalar.dma_start(out=t[:], in_=xr_d[m])
            xr_t.append(t)
        wt_t = consts.tile([128, 2, 2, 5, F], dt)
        nc.scalar.dma_start(out=wt_t[:], in_=wt_d[:])

        # stacked P tiles: [m][u]  u=0: [Pe_A;Po_A], u=1: [Po_B;Pe_B]
        stk = {(m, u): spool.tile([128, JCOLS], dt, name=f"stk{m}{u}",
                                  tag=f"stk{m}{u}")
               for m in range(2) for u in range(2)}

        t1t_ev = {}    # key (m-parity-free) e -> [32,128] rows 0:8 valid
        t1t_far = {}   # key (e, g) -> [32,128] rows 8i:8i+8 = chunk 4g+i

        def proj(m):
            # projections into psum: far cols [8c, 8c+8), even cols [128,136)
            for e in (1, 0):
                t1 = t1ps.tile([128, 128], f32, name=f"t1_{m}_{e}", tag="t1f")
                t1e = t1ps.tile([128, 8], f32, name=f"t1e_{m}_{e}", tag="t1e")
                for rt in range(16):
                    nc.tensor.matmul(t1e[0:64, 0:8],
                                     xeo_t[(m, e)][:, rt + 2, 64:128],
                                     ue_t[:, e, rt, :],
                                     start=(rt == 0), stop=(rt == 15))
                    nc.tensor.matmul(t1e[64:128, 0:8],
                                     xeo_t[(m, e)][:, rt + 2, 0:64],
                                     ue_t[:, e, rt, :],
                                     start=(rt == 0), stop=(rt == 15),
                                     skip_group_check=True)
                jp = 1 - e
                for si in range(16):
                    nc.tensor.matmul(t1[:, :],
                                     xeo_t[(m, e)][:, si + 2, :],
                                     uf_t[:, jp, si, :],
                                     start=(si == 0), stop=(si == 15))
                # transposes: even -> t1t_ev[e], far -> t1t_far[(e, g)]
                tc_ev = tpool.tile([128, 32], dt, name=f"t1cev_{m}_{e}", tag="t1c")
                nc.vector.tensor_copy(out=tc_ev[:, 0:8], in_=t1e[:, 0:8])
                nc.vector.memset(tc_ev[:, 8:32], 0.0)
                tt = tpool.tile([32, 128], dt, name=f"t1tev_{m}_{e}", tag=f"ttev{e}")
                for b in range(4):
                    nc.vector.transpose(out=tt[0:32, 32 * b:32 * b + 32],
                                        in_=tc_ev[32 * b:32 * b + 32, 0:32])
                t1t_ev[e] = tt
                for g in range(4):
                    tc_f = tpool.tile([128, 32], dt, name=f"t1cf_{m}_{e}_{g}",
                                      tag="t1c")
                    nc.vector.tensor_copy(out=tc_f[:, 0:32],
                                          in_=t1[:, 32 * g:32 * g + 32])
                    ttf = tpool.tile([32, 128], dt, name=f"t1tf_{m}_{e}_{g}",
                                     tag=f"ttf{e}{g}")
                    for b in range(4):
                        nc.vector.transpose(out=ttf[0:32, 32 * b:32 * b + 32],
                                            in_=tc_f[32 * b:32 * b + 32, 0:32])
                    t1t_far[(e, g)] = ttf

        def stage1(m):
            for c in range(NCH):
                ch = CH_LAST if c == NCH - 1 else CH
                j0 = CH * c
                for jp in (0, 1):
                    eo = 1 - jp
                    ps = ps1.tile([128, CH_LAST], f32, name=f"p_{m}_{c}_{jp}",
                                  tag="ps1")
                    bands = [b for b in range(BAND) if c + 1 + b not in PADS]
                    for k, b in enumerate(bands):
                        nc.tensor.matmul(ps[:, 0:ch],
                                         xeo_t[(m, eo)][:, c + 1 + b, :],
                                         dm_t[:, jp, b, 0:ch],
                                         start=(k == 0), stop=False)
                    nc.tensor.matmul(ps[:, 0:ch], t1t_ev[jp][0:8, :],
                                     ve_t[0:8, jp, j0:j0 + ch],
                                     start=False, stop=False)
                    g, i = c // 4, c % 4
                    nc.tensor.matmul(ps[:, 0:ch],
                                     t1t_far[(eo, g)][0:32, :],
                                     vf_t[0:32, jp, i, 0:ch],
                                     start=False, stop=True)
                    # partition-aligned psum->stacked copies (+0.5x delta)
                    # jp=0 psum rows [A|B]; jp=1 rows [B|A]
                    u_top = 0 if jp == 0 else 1     # rows 0:64 -> stk[m][u_top]
                    u_bot = 1 - u_top
                    nc.vector.tensor_add(stk[(m, u_top)][0:64, j0:j0 + ch],
                                         ps[0:64, 0:ch],
                                         xr_t[m][0:64, u_top, j0:j0 + ch])
                    nc.vector.tensor_add(stk[(m, u_bot)][64:128, j0:j0 + ch],
                                         ps[64:128, 0:ch],
                                         xr_t[m][64:128, u_bot, j0:j0 + ch])

        def stage2(m, us=(0, 1)):
            for u in us:
                item = 2 * m + u
                for tch in range(4):
                    oa = oapool.tile([128, 1024], dt, name=f"oa_{item}_{tch}",
                                     tag="oa")
                    for op in range(2):
                        p2 = ps2.tile([128, 512], f32, name=f"q_{item}_{op}_{tch}",
                                      tag="ps2")
                        for g in range(5):
                            j0 = 512 * tch + g
                            nc.tensor.matmul(p2[:, :], wt_t[:, u, op, g, :],
                                             stk[(m, u)][:, j0:j0 + 512],
                                             start=(g == 0), stop=(g == 4))
                        nc.scalar.activation(
                            oa[:, op:1024:2], p2[:, :],
                            mybir.ActivationFunctionType.Identity, bias=btile[:])
                    nc.scalar.dma_start(
                        out=out_d[item, :, 1024 * tch:1024 * (tch + 1)],
                        in_=oa[:, :])

        proj(0)
        stage1(0)
        proj(1)
        stage2(0)
        stage1(1)
        stage2(1)

    nc.compile()
    return nc


def _prep_inputs(x, w, b):
    dm, uf, vf, ue, ve = _consts()
    dmq = dm.astype(bf16)
    ufq = uf.astype(bf16)
    vfq = vf.astype(bf16)
    ueq = ue.astype(bf16)
    veq = ve.astype(bf16)

    # stage-2 weights wt[p, u, op, g, f]
    wq = w.transpose(1, 2, 0).astype(np.float32)   # [c, s, f]
    wt = np.zeros((128, 2, 2, 5, F), np.float32)
    for g in range(5):
        # out-even groups: g<4: taps (2g, 2g+1); g=4: tap 8 on Pe only
        pe_ev = wq[:, 2 * g, :] if g < 4 else wq[:, 8, :]
        po_ev = wq[:, 2 * g + 1, :] if g < 4 else None
        # out-odd groups: g=0: tap 0 on Po; g>=1: taps (2g-1 on Pe, 2g on Po)
        pe_od = wq[:, 2 * g - 1, :] if g >= 1 else None
        po_od = wq[:, 2 * g, :] if g >= 1 else wq[:, 0, :]
        # u=0: rows 0:64 = Pe rows, 64:128 = Po rows; u=1 swapped
        wt[0:64, 0, 0, g] = pe_ev
        if po_ev is not None:
            wt[64:128, 0, 0, g] = po_ev
        if pe_od is not None:
            wt[0:64, 0, 1, g] = pe_od
        wt[64:128, 0, 1, g] = po_od
        wt[64:128, 1, 0, g] = pe_ev
        if po_ev is not None:
            wt[0:64, 1, 0, g] = po_ev
        if pe_od is not None:
            wt[64:128, 1, 1, g] = pe_od
        wt[0:64, 1, 1, g] = po_od
    wtq = wt.astype(bf16)
    bias = np.ascontiguousarray(b.reshape(128, 1).astype(np.float32))

    in_maps = []
    for core in range(N_CORES):
        x4 = x[NPC * core:NPC * core + NPC]
        xeo = np.zeros((2, 2, 128, 16, 128), np.float32)
        xr = np.zeros((2, 128, 2, JCOLS), np.float32)
        for m in range(2):
            iA, iB = 2 * m, 2 * m + 1
            rows = {1: np.concatenate([x4[iA], x4[iB]], axis=0),
                    0: np.concatenate([x4[iB], x4[iA]], axis=0)}
            for e in (0, 1):
                xp = rows[e][:, e::2]              # [128, 2048]
                xeo[m, e] = xp.T.reshape(16, 128, 128).transpose(1, 0, 2)
            # delta adds: tileA=[Pe_A;Po_A], tileB=[Po_B;Pe_B]
            xpA = np.zeros((C, 2 * JCOLS + 8), np.float32)
            xpB = np.zeros((C, 2 * JCOLS + 8), np.float32)
            xpA[:, PAD:PAD + W] = 0.5 * x4[iA]
            xpB[:, PAD:PAD + W] = 0.5 * x4[iB]
            xr[m, 0:64, 0] = xpA[:, 0:2 * JCOLS:2]
            xr[m, 64:128, 0] = xpA[:, 1:2 * JCOLS:2]
            xr[m, 0:64, 1] = xpB[:, 1:2 * JCOLS:2]
            xr[m, 64:128, 1] = xpB[:, 0:2 * JCOLS:2]
        in_maps.append({
            "xeo": xeo.astype(bf16), "xr": xr.astype(bf16),
            "dm": dmq, "uf": ufq, "ue": ueq, "vf": vfq, "ve": veq,
            "wt": wtq, "bias": bias,
        })
    return in_maps


def run(x, w, b, trace=False):
    nc = build_nc()
    in_maps = _prep_inputs(x, w, b)
    res = run_bass_kernel_spmd(nc, in_maps, list(range(N_CORES)), trace=trace)
    out = np.empty((N, F, OUT_W), np.float32)
    for core in range(N_CORES):
        out[NPC * core:NPC * core + NPC] = \
            np.asarray(res.results[core]["out"]).astype(np.float32)
    return out, res


def kernel(x, w, b):
    x = np.asarray(x, dtype=np.float32)
    w = np.asarray(w, dtype=np.float32)
    b = np.asarray(b, dtype=np.float32)
    out, _ = run(x, w, b, trace=False)
    return out


# revision 11
# speedup vs baseline: 1.1238x; 1.1238x over previous
"""Conv1dFFT (truncated-spectrum FFT conv) on 8 trn2 cores — banded matmul form.

Math: out = irfft(trunc(rfft(xp)) * conj(trunc(rfft(wp))))[..., :W] + b on a ring
of size L. Truncation == ring convolution with the Dirichlet kernel D:

    out[n,f,t] = sum_c sum_s w[f,c,s] * P[n,c,t+s] + b[f]
    P[n,c,j]   = sum_tau x[n,c,tau] * D(j - PAD - tau),  D(d) = sin(pi*H*d/L)/(L*sin(pi*d/L))

Structure exploited (per j-parity jp, P columns compacted by parity):
  D = 0.5*delta  (DVE add of 0.5*x, in "xr")
    + odd part   (opposite-parity taus): BANDED dense matmul (3 tau-tiles around
                 the diagonal per 128-col chunk; Toeplitz => the same D data
                 serves every chunk and every core)
                 + rank-4 far-field correction (SVD of the smooth off-band part;
                 per-slot zero-padded U makes the projection one psum group)
    + even part  (same-parity taus, smooth): global rank-4 correction.

P is stored column-parity-split and batch-stacked: per m-tile (2 batch items
A,B) two SBUF tiles  tileA=[Pe_A;Po_A], tileB=[Po_B;Pe_B]  (the e=0/e=1 x row
orders are swapped host-side so every PSUM->SBUF copy is partition-aligned).
Stage-1 psums are 512-col "quads" (4 chunks, one accumulation group per bank)
so the PSUM->SBUF drain amortizes the DVE psum-access overhead.
Stage 2 contracts the 9 filter taps as 5 K=128 matmuls per output parity
directly on the stacked tiles - no shifted P2 copy needed.

Sharding: pure data-parallel over batch N: 4 items per core.
"""

from contextlib import ExitStack

import numpy as np
import ml_dtypes

import concourse.bass as bass
import concourse.tile as tile
from concourse import bacc, mybir
from concourse.bass_utils import run_bass_kernel_spmd

# ---- problem constants ----
N, C, W = 32, 64, 4096
F, WW = 128, 9
PAD = 4
OUT_W = W - WW + 1 + 2 * PAD                    # 4096
L = W + 2 * PAD + 2 * (WW - 1) + (OUT_W - 1)    # 8215
INIT_HALF = L // 2 + 1
IB = min(INIT_HALF - 1, int(INIT_HALF * 0.5) + 1)
HALF = INIT_HALF - IB
H = 2 * HALF - 1                                # 4105

# ---- scheme constants ----
N_CORES = 8
NPC = N // N_CORES          # 4 batch items per core
JCOLS = 2052                # per-parity compacted P columns used (max j = 4103)
NCH = 16                    # column chunks per parity
CH = 128                    # cols per chunk (chunk 15: 132 = 128 + 4 in tiny psum)
CH_LAST = JCOLS - (NCH - 1) * CH   # 132
NSLOT = 19                  # tau tile slots; real tiles 0..15 at slots 2..17
PADS = (0, 1, 18)           # zero slots
BAND = 3                    # near-band tau tiles per chunk: slots [c+1, c+4)
RF = 4                      # far-field rank
RE = 4                      # even-part rank
RELS = list(range(-16, 0)) + list(range(3, 18))  # far relative tile offsets

bf16 = ml_dtypes.bfloat16


def _dirichlet():
    d = np.arange(-(L + 8), L + 9, dtype=np.float64)
    with np.errstate(invalid="ignore", divide="ignore"):
        Dv = np.sin(np.pi * H * d / L) / (L * np.sin(np.pi * d / L))
    Dv[np.abs(d) % L == 0] = H / L

    def DD(dv):
        return Dv[dv + L + 8]

    return DD


_CONSTS = {}


def _consts():
    """Shared (core-independent) constant tensors: dm, Ufar, Vfar, Uev, Vev."""
    if "c" in _CONSTS:
        return _CONSTS["c"]
    DD = _dirichlet()
    rng = np.random.default_rng(0)
    jc = np.arange(CH_LAST)
    p = np.arange(128)

    # near banded D: d = 2jc + 2jp + 251 - 256b - 2p
    dm = np.zeros((128, 2, BAND, CH_LAST), np.float32)          # [p, jp, b, col]
    for jp in (0, 1):
        for b in range(BAND):
            dm[:, jp, b, :] = DD(2 * jc[None, :] + 2 * jp + 251
                                 - 256 * b - 2 * p[:, None])

    # far field: relative matrix SVD -> per-slot padded U (one psum group) +
    # global V replicated per i-position for the K=32 zero-padded VS form
    uf = np.zeros((128, 2, 16, RF * NCH), np.float32)  # [p, jp, slot-2, RF*c+r]
    vf = np.zeros((32, 2, 8, CH_LAST), np.float32)     # [row, jp, i, col]
    for jp in (0, 1):
        Fm = np.zeros((len(RELS) * 128, CH_LAST))
        for i, rho in enumerate(RELS):
            Fm[i * 128:(i + 1) * 128] = DD(2 * jc[None, :] + 2 * jp + 251
                                           - 256 * rho - 2 * p[:, None])
        G = rng.normal(size=(CH_LAST, 4 * RF))
        Q, _ = np.linalg.qr(Fm @ G)
        u, s, vt = np.linalg.svd(Q.T @ Fm, full_matrices=False)
        UU = (Q @ u[:, :RF]).reshape(len(RELS), 128, RF)
        for s_ in range(2, 18):
            for c in range(NCH):
                rho = s_ - (c + 1)
                if rho in RELS:
                    uf[:, jp, s_ - 2, RF * c:RF * c + RF] = UU[RELS.index(rho)]
        for i in range(8):
            vf[RF * i:RF * i + RF, jp, i, :] = s[:RF, None] * vt[:RF, :]

    # even part (same parity, 0.5*delta removed): global rank-4
    ue = np.zeros((128, 2, 16, RE), np.float32)                 # [p, jp, rt, r]
    ve = np.zeros((RE, 2, JCOLS), np.float32)                   # [r, jp, col]
    for jp in (0, 1):
        jg = 2 * np.arange(JCOLS) + jp
        tg = 2 * np.arange(2048) + jp
        dmat = jg[None, :] - 4 - tg[:, None]
        A = DD(dmat) - 0.5 * (dmat == 0)
        G = rng.normal(size=(A.shape[1], 4 * RE))
        Q, _ = np.linalg.qr(A @ G)
        u, s, vt = np.linalg.svd(Q.T @ A, full_matrices=False)
        ue[:, jp] = (Q @ u[:, :RE]).reshape(16, 128, RE).transpose(1, 0, 2)
        ve[:, jp, :] = s[:RE, None] * vt[:RE, :]

    _CONSTS["c"] = (dm, uf, vf, ue, ve)
    return _CONSTS["c"]


def build_nc():
    dt = mybir.dt.bfloat16
    f32 = mybir.dt.float32
    nc = bacc.Bacc("TRN2", target_bir_lowering=False, debug=False)

    xeo_d = nc.dram_tensor("xeo", [2, 2, 128, 16, 128], dt, kind="ExternalInput")
    xr_d = nc.dram_tensor("xr", [2, 128, 2, JCOLS], dt, kind="ExternalInput")
    dm_d = nc.dram_tensor("dm", [128, 2, BAND, CH_LAST], dt, kind="ExternalInput")
    uf_d = nc.dram_tensor("uf", [128, 2, 16, RF * NCH], dt, kind="ExternalInput")
    ue_d = nc.dram_tensor("ue", [128, 2, 16, RE], dt, kind="ExternalInput")
    vf_d = nc.dram_tensor("vf", [32, 2, 8, CH_LAST], dt, kind="ExternalInput")
    ve_d = nc.dram_tensor("ve", [RE, 2, JCOLS], dt, kind="ExternalInput")
    wt_d = nc.dram_tensor("wt", [128, 2, 2, 5, F], dt, kind="ExternalInput")
    b_d = nc.dram_tensor("bias", [128, 1], f32, kind="ExternalInput")
    out_d = nc.dram_tensor("out", [NPC, F, OUT_W], dt, kind="ExternalOutput")

    with tile.TileContext(nc) as tc, ExitStack() as ctx:
        consts = ctx.enter_context(tc.tile_pool(name="consts", bufs=1))
        xpool = ctx.enter_context(tc.tile_pool(name="x", bufs=1))
        spool = ctx.enter_context(tc.tile_pool(name="st", bufs=1))
        tpool = ctx.enter_context(tc.tile_pool(name="t1", bufs=2))
        oapool = ctx.enter_context(tc.tile_pool(name="oa", bufs=3))
        t1ps = ctx.enter_context(tc.tile_pool(name="t1ps", bufs=1, space="PSUM"))
        ps1 = ctx.enter_context(tc.tile_pool(name="ps1", bufs=3, space="PSUM"))
        ps2 = ctx.enter_context(tc.tile_pool(name="ps2", bufs=3, space="PSUM"))

        # ---- loads; sync queue: dm, ue, xeo-m0, uf, xeo-m1 ----
        dm_t = consts.tile([128, 2, BAND, CH_LAST], dt)
        nc.sync.dma_start(out=dm_t[:], in_=dm_d[:])
        ue_t = consts.tile([128, 2, 16, RE], dt)
        nc.sync.dma_start(out=ue_t[:], in_=ue_d[:])

        xeo_t = {}
        for m in range(2):
            for e in (1, 0):
                t = xpool.tile([128, NSLOT, 128], dt, name=f"xeo{m}{e}",
                               tag=f"xeo{m}{e}")
                nc.gpsimd.memset(t[:, 0:2, :], 0.0)
                nc.gpsimd.memset(t[:, 18:19, :], 0.0)
                xeo_t[(m, e)] = t

        def load_xeo(m):
            for e in (1, 0):
                t = xeo_t[(m, e)]
                nc.sync.dma_start(out=t[:, 2:10, :], in_=xeo_d[m, e, :, 0:8])
                nc.sync.dma_start(out=t[:, 10:18, :], in_=xeo_d[m, e, :, 8:16])

        load_xeo(0)
        uf_t = consts.tile([128, 2, 16, RF * NCH], dt)
        nc.sync.dma_start(out=uf_t[:], in_=uf_d[:])
        load_xeo(1)

        # scalar queue: xr-m0, vf, ve, bias, wt, xr-m1
        xr_t = []
        for m in range(2):
            t = xpool.tile([128, 2, JCOLS], dt, name=f"xr{m}", tag=f"xr{m}")
            xr_t.append(t)
        nc.scalar.dma_start(out=xr_t[0][:], in_=xr_d[0])
        vf_t = consts.tile([32, 2, 8, CH_LAST], dt)
        nc.scalar.dma_start(out=vf_t[:], in_=vf_d[:])
        ve_t = consts.tile([RE, 2, JCOLS], dt)
        nc.scalar.dma_start(out=ve_t[:], in_=ve_d[:])
        btile = consts.tile([128, 1], f32)
        nc.scalar.dma_start(out=btile[:], in_=b_d[:])
        wt_t = consts.tile([128, 2, 2, 5, F], dt)
        nc.scalar.dma_start(out=wt_t[:], in_=wt_d[:])
        nc.scalar.dma_start(out=xr_t[1][:], in_=xr_d[1])

        # stacked P tiles: [m][u]  u=0: [Pe_A;Po_A], u=1: [Po_B;Pe_B]
        stk = {(m, u): spool.tile([128, JCOLS], dt, name=f"stk{m}{u}",
                                  tag=f"stk{m}{u}")
               for m in range(2) for u in range(2)}

        t1t_ev = {}    # e -> [32,128], rows 0:RE valid
        t1t_far = {}   # (e, g) -> [32,128], rows RF*(c%8) for chunk c, g=c//8

        def proj(m):
            for e in (1, 0):
                t1 = t1ps.tile([128, RF * NCH], f32, name=f"t1_{m}_{e}", tag="t1f")
                t1e = t1ps.tile([128, RE], f32, name=f"t1e_{m}_{e}", tag="t1e")
                for rt in range(16):
                    nc.tensor.matmul(t1e[0:64, 0:RE],
                                     xeo_t[(m, e)][:, rt + 2, 64:128],
                                     ue_t[:, e, rt, :],
                                     start=(rt == 0), stop=(rt == 15))
                    nc.tensor.matmul(t1e[64:128, 0:RE],
                                     xeo_t[(m, e)][:, rt + 2, 0:64],
                                     ue_t[:, e, rt, :],
                                     start=(rt == 0), stop=(rt == 15),
                                     skip_group_check=True)
                jp = 1 - e
                for si in range(16):
                    nc.tensor.matmul(t1[:, :],
                                     xeo_t[(m, e)][:, si + 2, :],
                                     uf_t[:, jp, si, :],
                                     start=(si == 0), stop=(si == 15))
                # transposes; psum->sbuf copies on the Activation engine
                tc_ev = tpool.tile([128, 32], dt, name=f"t1cev_{m}_{e}", tag="t1c")
                nc.scalar.activation(tc_ev[:, 0:RE], t1e[:, 0:RE],
                                     mybir.ActivationFunctionType.Copy)
                nc.vector.memset(tc_ev[:, RE:32], 0.0)
                tt = tpool.tile([32, 128], dt, name=f"t1tev_{m}_{e}", tag=f"ttev{e}")
                for b in range(4):
                    nc.vector.transpose(out=tt[0:32, 32 * b:32 * b + 32],
                                        in_=tc_ev[32 * b:32 * b + 32, 0:32])
                t1t_ev[e] = tt
                for g in range(2):
                    tc_f = tpool.tile([128, 32], dt, name=f"t1cf_{m}_{e}_{g}",
                                      tag="t1c")
                    nc.scalar.activation(tc_f[:, 0:32], t1[:, 32 * g:32 * g + 32],
                                         mybir.ActivationFunctionType.Copy)
                    ttf = tpool.tile([32, 128], dt, name=f"t1tf_{m}_{e}_{g}",
                                     tag=f"ttf{e}{g}")
                    for b in range(4):
                        nc.vector.transpose(out=ttf[0:32, 32 * b:32 * b + 32],
                                            in_=tc_f[32 * b:32 * b + 32, 0:32])
                    t1t_far[(e, g)] = ttf

        def chunk_mms(m, jp, ps, c, pcol, ccol, w, first, last):
            """Near + VS matmuls for chunk c into ps[:, pcol:pcol+w], using
            dm/vf cols [ccol, ccol+w). One psum group spans the whole tile:
            start only on the quad's first matmul, stop on its last."""
            eo = 1 - jp
            bands = [b for b in range(BAND) if c + 1 + b not in PADS]
            for k, b in enumerate(bands):
                st = first and k == 0
                nc.tensor.matmul(ps[:, pcol:pcol + w],
                                 xeo_t[(m, eo)][:, c + 1 + b, :],
                                 dm_t[:, jp, b, ccol:ccol + w],
                                 start=st, stop=False)
            nc.tensor.matmul(ps[:, pcol:pcol + w], t1t_ev[jp][0:RE, :],
                             ve_t[0:RE, jp, 128 * c + ccol:128 * c + ccol + w],
                             start=False, stop=False)
            g, i = c // 8, c % 8
            nc.tensor.matmul(ps[:, pcol:pcol + w],
                             t1t_far[(eo, g)][0:32, :],
                             vf_t[0:32, jp, i, ccol:ccol + w],
                             start=False, stop=last)

        def copy_adds(m, jp, ps, j0, wdt, pcol=0):
            u_top = 0 if jp == 0 else 1
            u_bot = 1 - u_top
            nc.vector.tensor_add(stk[(m, u_top)][0:64, j0:j0 + wdt],
                                 ps[0:64, pcol:pcol + wdt],
                                 xr_t[m][0:64, u_top, j0:j0 + wdt])
            nc.vector.tensor_add(stk[(m, u_bot)][64:128, j0:j0 + wdt],
                                 ps[64:128, pcol:pcol + wdt],
                                 xr_t[m][64:128, u_bot, j0:j0 + wdt])

        def stage1(m):
            for q in range(4):
                for jp in (0, 1):
                    ps = ps1.tile([128, 512], f32, name=f"p_{m}_{q}_{jp}",
                                  tag="ps1")
                    for k in range(4):
                        chunk_mms(m, jp, ps, 4 * q + k, 128 * k, 0, 128,
                                  first=(k == 0), last=(k == 3))
                    copy_adds(m, jp, ps, 512 * q, 512)
            # tiny psum: chunk 15 cols 128:132 -> global cols 2048:2052
            for jp in (0, 1):
                ps = ps1.tile([128, 512], f32, name=f"pt_{m}_{jp}", tag="ps1")
                chunk_mms(m, jp, ps, 15, 0, 128, 4, first=True, last=True)
                copy_adds(m, jp, ps, 2048, 4)

        def stage2(m, us=(0, 1)):
            for u in us:
                item = 2 * m + u
                for tch in range(4):
                    oa = oapool.tile([128, 1024], dt, name=f"oa_{item}_{tch}",
                                     tag="oa")
                    for op in range(2):
                        p2 = ps2.tile([128, 512], f32, name=f"q_{item}_{op}_{tch}",
                                      tag="ps2")
                        for g in range(5):
                            j0 = 512 * tch + g
                            nc.tensor.matmul(p2[:, :], wt_t[:, u, op, g, :],
                                             stk[(m, u)][:, j0:j0 + 512],
                                             start=(g == 0), stop=(g == 4))
                        nc.scalar.activation(
                            oa[:, op:1024:2], p2[:, :],
                            mybir.ActivationFunctionType.Identity, bias=btile[:])
                    nc.sync.dma_start(
                        out=out_d[item, :, 1024 * tch:1024 * (tch + 1)],
                        in_=oa[:, :])

        proj(0)
        stage1(0)
        proj(1)
        stage2(0)
        stage1(1)
        stage2(1)

    nc.compile()
    return nc


def _prep_inputs(x, w, b):
    dm, uf, vf, ue, ve = _consts()
    dmq = dm.astype(bf16)
    ufq = uf.astype(bf16)
    vfq = vf.astype(bf16)
    ueq = ue.astype(bf16)
    veq = ve.astype(bf16)

    # stage-2 weights wt[p, u, op, g, f]
    wq = w.transpose(1, 2, 0).astype(np.float32)   # [c, s, f]
    wt = np.zeros((128, 2, 2, 5, F), np.float32)
    for g in range(5):
        pe_ev = wq[:, 2 * g, :] if g < 4 else wq[:, 8, :]
        po_ev = wq[:, 2 * g + 1, :] if g < 4 else None
        pe_od = wq[:, 2 * g - 1, :] if g >= 1 else None
        po_od = wq[:, 2 * g, :] if g >= 1 else wq[:, 0, :]
        wt[0:64, 0, 0, g] = pe_ev
        if po_ev is not None:
            wt[64:128, 0, 0, g] = po_ev
        if pe_od is not None:
            wt[0:64, 0, 1, g] = pe_od
        wt[64:128, 0, 1, g] = po_od
        wt[64:128, 1, 0, g] = pe_ev
        if po_ev is not None:
            wt[0:64, 1, 0, g] = po_ev
        if pe_od is not None:
            wt[64:128, 1, 1, g] = pe_od
        wt[0:64, 1, 1, g] = po_od
    wtq = wt.astype(bf16)
    bias = np.ascontiguousarray(b.reshape(128, 1).astype(np.float32))

    in_maps = []
    for core in range(N_CORES):
        x4 = x[NPC * core:NPC * core + NPC]
        xeo = np.zeros((2, 2, 128, 16, 128), np.float32)
        xr = np.zeros((2, 128, 2, JCOLS), np.float32)
        for m in range(2):
            iA, iB = 2 * m, 2 * m + 1
            rows = {1: np.concatenate([x4[iA], x4[iB]], axis=0),
                    0: np.concatenate([x4[iB], x4[iA]], axis=0)}
            for e in (0, 1):
                xp = rows[e][:, e::2]              # [128, 2048]
                xeo[m, e] = xp.T.reshape(16, 128, 128).transpose(1, 0, 2)
            xpA = np.zeros((C, 2 * JCOLS + 8), np.float32)
            xpB = np.zeros((C, 2 * JCOLS + 8), np.float32)
            xpA[:, PAD:PAD + W] = 0.5 * x4[iA]
            xpB[:, PAD:PAD + W] = 0.5 * x4[iB]
            xr[m, 0:64, 0] = xpA[:, 0:2 * JCOLS:2]
            xr[m, 64:128, 0] = xpA[:, 1:2 * JCOLS:2]
            xr[m, 0:64, 1] = xpB[:, 1:2 * JCOLS:2]
            xr[m, 64:128, 1] = xpB[:, 0:2 * JCOLS:2]
        in_maps.append({
            "xeo": xeo.astype(bf16), "xr": xr.astype(bf16),
            "dm": dmq, "uf": ufq, "ue": ueq, "vf": vfq, "ve": veq,
            "wt": wtq, "bias": bias,
        })
    return in_maps


def run(x, w, b, trace=False):
    nc = build_nc()
    in_maps = _prep_inputs(x, w, b)
    res = run_bass_kernel_spmd(nc, in_maps, list(range(N_CORES)), trace=trace)
    out = np.empty((N, F, OUT_W), np.float32)
    for core in range(N_CORES):
        out[NPC * core:NPC * core + NPC] = \
            np.asarray(res.results[core]["out"]).astype(np.float32)
    return out, res


def kernel(x, w, b):
    x = np.asarray(x, dtype=np.float32)
    w = np.asarray(w, dtype=np.float32)
    b = np.asarray(b, dtype=np.float32)
    out, _ = run(x, w, b, trace=False)
    return out
